# revision 21
# baseline (speedup 1.0000x reference)
"""Trainium2 Bass kernel for nn_CompositeHeadB (composite octree head).

The model is: per depth d in 1..5, slice x -> head_d(x_slice), concat tokens.
Heads 1-3 are Linear(E, V1); heads 4-5 are ConvTranspose1d(E, E, k, stride=k)
followed by Linear(E, V1). Because the conv is stride=k with kernel k (non
overlapping), every output token (latent t, sub-position j) is
    out[t*k + j, :] = x[t, :] @ (K[:, :, j] @ W) + (cb @ W + b)
i.e. the whole network folds into 13 small [E, V1] matrices applied to the
latent vectors. The kernel is therefore a memory-bound streaming matmul:
read x once (52.7 MB), write logits (20.2 MB).

Sharding: 8 cores = 4 batches x 2 halves of every depth's latent range.
Each core's x slice is fed pre-transposed as [E, T_core] so the contraction
dim (E) lands on SBUF partitions with no on-device transpose. Outputs are
produced transposed, [r*V1, T_core_d] per depth, and un-transposed on the
host during the gather.
"""

import sys

for _p in ("/opt/trn_rl_repo",):
    if _p not in sys.path:
        sys.path.append(_p)

import numpy as np

P = 128
E = 256
V1 = 17
COUNTS = (8, 64, 512, 8192, 65536)
RED = (1, 1, 1, 2, 8)
LAT = tuple(c // r for c, r in zip(COUNTS, RED))  # (8, 64, 512, 4096, 8192)
LOFFS = np.cumsum((0,) + LAT)  # latent offsets per depth in x
TOKOFFS = np.cumsum((0,) + COUNTS)  # token offsets per depth in the output
N_BATCH = 4
N_CORES = 8
HALF = [l // 2 for l in LAT]  # per-core latent counts: [4, 32, 256, 2048, 4096]
T_CORE = sum(HALF)  # 6436

# Core-local column layout: depths ordered 5,4,3,2,1 (0-based idx 4..0).
CORE_ORDER = (4, 3, 2, 1, 0)
SEG_OFF = {4: 0, 3: 4096, 2: 6144, 1: 6400, 0: 6432}

# Folded weight matrix Mcat [E, 221]: cols = [M5 (136) | M4 (34) | M3 | M2 | M1].
# The [68, 6] bias table rides along as 6 extra columns of the o=0 chunk so
# weights+bias arrive in 2 DMAs total (stay within the 8 HWDGE sem lanes).
MC_W = 221
MC_WE = 227
# Matmul groups: (out_name, depth_idx, mcat_col_off, width, bias_col, t0, T)
GROUPS = (
    ("o5a", 4, 0, 68, 0, 0, 4096),
    ("o5b", 4, 68, 68, 1, 0, 4096),
    ("o4", 3, 136, 34, 2, 4096, 2048),
    ("o3", 2, 170, 17, 3, 6144, 256),
    ("o2", 1, 187, 17, 4, 6400, 32),
    ("o1", 0, 204, 17, 5, 6432, 4),
)
# DMA load blocks over core columns (start, width).
BLOCKS = ((0, 2048), (2048, 2048), (4096, 2048), (6144, 292))
MM_SLICE = 512  # max fp32 moving free dim

MM_DTYPE = "f32"  # "f32" (exact) | "f32r_native" (full PE rate, tf32-ish rounding)

_prog = None


def build_program():
    import concourse.bass as bass
    import concourse.mybir as mybir
    import concourse.tile as tile
    from concourse import bacc
    from concourse.tile_rust import add_dep_helper

    f32 = mybir.dt.float32
    f32r = mybir.dt.float32r
    in_dt = f32r if MM_DTYPE == "f32r_native" else f32

    def mm_ap(ap):
        return ap.bitcast(f32r) if MM_DTYPE == "f32r" else ap

    nc = bacc.Bacc(
        "TRN2",
        target_bir_lowering=False,
        debug=False,
        enable_asserts=False,
        num_devices=N_CORES,
    )
    xd = nc.dram_tensor("x", [2, P, T_CORE], in_dt, kind="ExternalInput").ap()
    md = nc.dram_tensor("mc", [2, P, MC_WE], in_dt, kind="ExternalInput").ap()
    outs = {}
    for name, _di, _mo, w, _bc, _t0, T in GROUPS:
        outs[name] = nc.dram_tensor(name, [w, T], f32, kind="ExternalOutput").ap()

    with tile.TileContext(nc) as tc:
        with (
            tc.tile_pool(name="wpool", bufs=1) as wpool,
            tc.tile_pool(name="xpool", bufs=4) as xpool,
            tc.tile_pool(name="opool", bufs=8) as opool,
            tc.tile_pool(name="ppool", bufs=6, space="PSUM") as ppool,
            tc.tile_pool(name="psync", bufs=1, space="PSUM") as psync,
        ):
            # fp32/fp32r matmuls lower to a self-loading LDW+MM pair whose
            # instruction struct holds at most ONE sync-wait. A matmul whose
            # weights tile AND moving tile both arrive by DMA would need two
            # waits and fails walrus codegen ("Too many sync wait commands").
            # Absorber: a 1-column dummy matmul per DMA-produced input tile
            # takes that tile's wait; real matmuls then see the dep already
            # observed on the PE vector clock (ordering edges, no semaphore).
            sync_ps = psync.tile([1, 16], f32, tag="sync")
            sync_k = [0]

            def pe_absorb(t_ap):
                k = sync_k[0]
                sync_k[0] += 1
                return nc.tensor.matmul(
                    sync_ps[:, k : k + 1],
                    mm_ap(t_ap[:, 0:1]),
                    mm_ap(t_ap[:, 0:1]),
                    start=True,
                    stop=True,
                )

            mt = []
            mt_abs = []
            for o in range(2):
                t = wpool.tile([P, MC_WE], in_dt, tag=f"mc{o}")
                nc.sync.dma_start(t[:], md[o, :, :])
                mt.append(t)
                mt_abs.append(pe_absorb(t))
            bt = mt[0][0:68, MC_W : MC_W + 6].bitcast(f32)
            # Absorb the weights/bias DMA wait on DVE (TensorScalarPtr also
            # holds only one sync-wait; real copies must only wait on PE).
            dve_scratch = wpool.tile([1, 1], f32, tag="dvesync")
            nc.vector.tensor_copy(dve_scratch[:], bt[0:1, 0:1])

            for b0, bw in BLOCKS:
                xt = xpool.tile([P, 2, bw], in_dt, tag="x")
                nc.sync.dma_start(xt[:], xd[:, :, b0 : b0 + bw].rearrange("o p t -> p o t"))
                x_abs = pe_absorb(xt[:, 0])
                for name, _di, mo, w, bc, t0, T in GROUPS:
                    g0 = max(b0, t0)
                    g1 = min(b0 + bw, t0 + T)
                    if g0 >= g1:
                        continue
                    ow = g1 - g0
                    ot = opool.tile([w, ow], f32, tag="ot")
                    for s0 in range(g0, g1, MM_SLICE):
                        sw = min(MM_SLICE, g1 - s0)
                        ps = ppool.tile([w, sw], f32)
                        for o in range(2):
                            mm = nc.tensor.matmul(
                                ps[:],
                                mm_ap(mt[o][:, mo : mo + w]),
                                mm_ap(xt[:, o, s0 - b0 : s0 - b0 + sw]),
                                start=(o == 0),
                                stop=(o == 1),
                            )
                            add_dep_helper(
                                mm.ins, mt_abs[o].ins, sync=False, reason="absorb mc wait"
                            )
                            add_dep_helper(
                                mm.ins, x_abs.ins, sync=False, reason="absorb x wait"
                            )
                        nc.vector.tensor_scalar_add(
                            ot[:, s0 - g0 : s0 - g0 + sw], ps[:], bt[:w, bc : bc + 1]
                        )
                    # SWDGE for stores: separate 8-lane sem pool from the
                    # HWDGE input DMAs, so no DMA ever needs a lane-reuse
                    # wait on top of its data wait.
                    nc.gpsimd.dma_start(outs[name][:, g0 - t0 : g1 - t0], ot[:])
    nc.compile()
    return nc


def _get_prog():
    global _prog
    if _prog is None:
        _prog = build_program()
    return _prog


def shard_inputs(inputs):
    """Full inputs -> per-core in_maps (host-side layout prep + weight fold)."""
    x = np.asarray(inputs["x"], np.float32)
    W = [np.asarray(inputs[f"W{i}"], np.float32) for i in (1, 2, 3, 4, 5)]
    b = [np.asarray(inputs[f"b{i}"], np.float32) for i in (1, 2, 3, 4, 5)]
    K4 = np.asarray(inputs["K4"], np.float32)
    cb4 = np.asarray(inputs["cb4"], np.float32)
    K5 = np.asarray(inputs["K5"], np.float32)
    cb5 = np.asarray(inputs["cb5"], np.float32)

    M5 = np.einsum("iok,ov->ikv", K5, W[4]).reshape(E, 8 * V1)
    M4 = np.einsum("iok,ov->ikv", K4, W[3]).reshape(E, 2 * V1)
    mcat = np.concatenate([M5, M4, W[2], W[1], W[0]], axis=1)
    assert mcat.shape == (E, MC_W)

    c5 = np.tile(cb5 @ W[4] + b[4], 8)  # [136]
    c4 = np.tile(cb4 @ W[3] + b[3], 2)  # [34]
    bias = np.zeros((68, 6), np.float32)
    bias[:68, 0] = c5[:68]
    bias[:68, 1] = c5[68:]
    bias[:34, 2] = c4
    bias[:17, 3] = b[2]
    bias[:17, 4] = b[1]
    bias[:17, 5] = b[0]

    mc = np.zeros((2, P, MC_WE), np.float32)
    mc[:, :, :MC_W] = mcat.reshape(2, P, MC_W)
    mc[0, :68, MC_W:] = bias

    in_maps = []
    for c in range(N_CORES):
        n, h = divmod(c, 2)
        parts = []
        for di in CORE_ORDER:
            lo = LOFFS[di] + h * HALF[di]
            parts.append(x[n, lo : lo + HALF[di]].T)
        xt = np.ascontiguousarray(np.concatenate(parts, axis=1), np.float32)
        in_maps.append({"x": xt.reshape(2, P, T_CORE), "mc": mc})
    return in_maps


def assemble_output(results):
    """Per-core result dicts -> full [N, T_TOK, V1] output."""
    out = np.empty((N_BATCH, int(TOKOFFS[-1]), V1), np.float32)
    for c in range(N_CORES):
        n, h = divmod(c, 2)
        r = results[c]
        per_depth = {
            4: np.concatenate([r["o5a"], r["o5b"]], axis=0),
            3: r["o4"],
            2: r["o3"],
            1: r["o2"],
            0: r["o1"],
        }
        for di in range(5):
            red = RED[di]
            th = HALF[di]  # latents in this half
            arr = per_depth[di]  # [red*V1, th]
            a = arr.reshape(red, V1, th).transpose(2, 0, 1).reshape(th * red, V1)
            t0 = int(TOKOFFS[di]) + h * (COUNTS[di] // 2)
            out[n, t0 : t0 + th * red] = a
    return out


def run_spmd(in_maps, trace=False, trace_kwargs=None):
    from concourse import bass_utils

    nc = _get_prog()
    return bass_utils.run_bass_kernel_spmd(
        nc,
        in_maps,
        list(range(N_CORES)),
        trace=trace,
        **(trace_kwargs or {}),
    )


def kernel(**inputs):
    res = run_spmd(shard_inputs(inputs))
    return assemble_output(res.results)


# revision 23
# speedup vs baseline: 1.4793x; 1.4793x over previous
"""Trainium2 Bass kernel for nn_CompositeHeadB (composite octree head).

The model is: per depth d in 1..5, slice x -> head_d(x_slice), concat tokens.
Heads 1-3 are Linear(E, V1); heads 4-5 are ConvTranspose1d(E, E, k, stride=k)
followed by Linear(E, V1). Because the conv is stride=k with kernel k (non
overlapping), every output token (latent t, sub-position j) is
    out[t*k + j, :] = x[t, :] @ (K[:, :, j] @ W) + (cb @ W + b)
i.e. the whole network folds into 13 small [E, V1] matrices applied to the
latent vectors. The kernel is therefore a memory-bound streaming matmul:
read x once (52.7 MB), write logits (20.2 MB).

Sharding: 8 cores = 4 batches x 2 halves of every depth's latent range.
Each core's x slice is fed pre-transposed as [E, T_core] so the contraction
dim (E) lands on SBUF partitions with no on-device transpose. Outputs are
produced transposed, [r*V1, T_core_d] per depth, and un-transposed on the
host during the gather.
"""

import sys

for _p in ("/opt/trn_rl_repo",):
    if _p not in sys.path:
        sys.path.append(_p)

import numpy as np

P = 128
E = 256
V1 = 17
COUNTS = (8, 64, 512, 8192, 65536)
RED = (1, 1, 1, 2, 8)
LAT = tuple(c // r for c, r in zip(COUNTS, RED))  # (8, 64, 512, 4096, 8192)
LOFFS = np.cumsum((0,) + LAT)  # latent offsets per depth in x
TOKOFFS = np.cumsum((0,) + COUNTS)  # token offsets per depth in the output
N_BATCH = 4
N_CORES = 8
HALF = [l // 2 for l in LAT]  # per-core latent counts: [4, 32, 256, 2048, 4096]
T_CORE = sum(HALF)  # 6436

# Core-local column layout: depths ordered 5,4,3,2,1 (0-based idx 4..0).
CORE_ORDER = (4, 3, 2, 1, 0)
SEG_OFF = {4: 0, 3: 4096, 2: 6144, 1: 6400, 0: 6432}

# Folded weight matrix Mcat [E, 221]: cols = [M5 (136) | M4 (34) | M3 | M2 | M1].
# The [68, 6] bias table rides along as 6 extra columns of the o=0 chunk so
# weights+bias arrive in 2 DMAs total (stay within the 8 HWDGE sem lanes).
MC_W = 221
MC_WE = 227
# Matmul groups: (out_name, depth_idx, mcat_col_off, width, bias_col, t0, T)
GROUPS = (
    ("o5a", 4, 0, 68, 0, 0, 4096),
    ("o5b", 4, 68, 68, 1, 0, 4096),
    ("o4", 3, 136, 34, 2, 4096, 2048),
    ("o3", 2, 170, 17, 3, 6144, 256),
    ("o2", 1, 187, 17, 4, 6400, 32),
    ("o1", 0, 204, 17, 5, 6432, 4),
)
# DMA load blocks over core columns (start, width).
BLOCKS = ((0, 2048), (2048, 2048), (4096, 2048), (6144, 292))
MM_SLICE = 512  # max fp32 moving free dim

MM_DTYPE = "f32r_native"

_prog = None


def build_program():
    import concourse.bass as bass
    import concourse.mybir as mybir
    import concourse.tile as tile
    from concourse import bacc
    from concourse.tile_rust import add_dep_helper

    f32 = mybir.dt.float32
    f32r = mybir.dt.float32r
    in_dt = f32r if MM_DTYPE == "f32r_native" else f32

    def mm_ap(ap):
        return ap.bitcast(f32r) if MM_DTYPE == "f32r" else ap

    nc = bacc.Bacc(
        "TRN2",
        target_bir_lowering=False,
        debug=False,
        enable_asserts=False,
        num_devices=N_CORES,
    )
    xd = nc.dram_tensor("x", [2, P, T_CORE], in_dt, kind="ExternalInput").ap()
    md = nc.dram_tensor("mc", [2, P, MC_WE], in_dt, kind="ExternalInput").ap()
    outs = {}
    for name, _di, _mo, w, _bc, _t0, T in GROUPS:
        outs[name] = nc.dram_tensor(name, [w, T], f32, kind="ExternalOutput").ap()

    with tile.TileContext(nc) as tc:
        with (
            tc.tile_pool(name="wpool", bufs=1) as wpool,
            tc.tile_pool(name="xpool", bufs=4) as xpool,
            tc.tile_pool(name="opool", bufs=8) as opool,
            tc.tile_pool(name="ppool", bufs=6, space="PSUM") as ppool,
            tc.tile_pool(name="psync", bufs=1, space="PSUM") as psync,
        ):
            # fp32/fp32r matmuls lower to a self-loading LDW+MM pair whose
            # instruction struct holds at most ONE sync-wait. A matmul whose
            # weights tile AND moving tile both arrive by DMA would need two
            # waits and fails walrus codegen ("Too many sync wait commands").
            # Absorber: a 1-column dummy matmul per DMA-produced input tile
            # takes that tile's wait; real matmuls then see the dep already
            # observed on the PE vector clock (ordering edges, no semaphore).
            sync_ps = psync.tile([1, 16], f32, tag="sync")
            sync_k = [0]

            def pe_absorb(t_ap):
                k = sync_k[0]
                sync_k[0] += 1
                # always plain f32 (1-wide f32r matmuls fail the ISA check)
                return nc.tensor.matmul(
                    sync_ps[:, k : k + 1],
                    t_ap[:, 0:1].bitcast(f32),
                    t_ap[:, 0:1].bitcast(f32),
                    start=True,
                    stop=True,
                )

            mt = []
            mt_abs = []
            for o in range(2):
                t = wpool.tile([P, MC_WE], in_dt, tag=f"mc{o}")
                nc.sync.dma_start(t[:], md[o, :, :])
                mt.append(t)
                mt_abs.append(pe_absorb(t))
            bt = mt[0][0:68, MC_W : MC_W + 6].bitcast(f32)
            # Absorb the weights/bias DMA wait on DVE (TensorScalarPtr also
            # holds only one sync-wait; real copies must only wait on PE).
            dve_scratch = wpool.tile([1, 1], f32, tag="dvesync")
            nc.vector.tensor_copy(dve_scratch[:], bt[0:1, 0:1])

            for b0, bw in BLOCKS:
                xt = xpool.tile([P, 2, bw], in_dt, tag="x")
                nc.sync.dma_start(xt[:], xd[:, :, b0 : b0 + bw].rearrange("o p t -> p o t"))
                x_abs = pe_absorb(xt[:, 0])
                for name, _di, mo, w, bc, t0, T in GROUPS:
                    g0 = max(b0, t0)
                    g1 = min(b0 + bw, t0 + T)
                    if g0 >= g1:
                        continue
                    ow = g1 - g0
                    ot = opool.tile([w, ow], f32, tag="ot")
                    for s0 in range(g0, g1, MM_SLICE):
                        sw = min(MM_SLICE, g1 - s0)
                        ps = ppool.tile([w, sw], f32)
                        for o in range(2):
                            mm = nc.tensor.matmul(
                                ps[:],
                                mm_ap(mt[o][:, mo : mo + w]),
                                mm_ap(xt[:, o, s0 - b0 : s0 - b0 + sw]),
                                start=(o == 0),
                                stop=(o == 1),
                            )
                            add_dep_helper(
                                mm.ins, mt_abs[o].ins, sync=False, reason="absorb mc wait"
                            )
                            add_dep_helper(
                                mm.ins, x_abs.ins, sync=False, reason="absorb x wait"
                            )
                        nc.vector.tensor_scalar_add(
                            ot[:, s0 - g0 : s0 - g0 + sw], ps[:], bt[:w, bc : bc + 1]
                        )
                    # SWDGE for stores: separate 8-lane sem pool from the
                    # HWDGE input DMAs, so no DMA ever needs a lane-reuse
                    # wait on top of its data wait.
                    nc.gpsimd.dma_start(outs[name][:, g0 - t0 : g1 - t0], ot[:])
    nc.compile()
    return nc


def _get_prog():
    global _prog
    if _prog is None:
        _prog = build_program()
    return _prog


def shard_inputs(inputs):
    """Full inputs -> per-core in_maps (host-side layout prep + weight fold)."""
    x = np.asarray(inputs["x"], np.float32)
    W = [np.asarray(inputs[f"W{i}"], np.float32) for i in (1, 2, 3, 4, 5)]
    b = [np.asarray(inputs[f"b{i}"], np.float32) for i in (1, 2, 3, 4, 5)]
    K4 = np.asarray(inputs["K4"], np.float32)
    cb4 = np.asarray(inputs["cb4"], np.float32)
    K5 = np.asarray(inputs["K5"], np.float32)
    cb5 = np.asarray(inputs["cb5"], np.float32)

    M5 = np.einsum("iok,ov->ikv", K5, W[4]).reshape(E, 8 * V1)
    M4 = np.einsum("iok,ov->ikv", K4, W[3]).reshape(E, 2 * V1)
    mcat = np.concatenate([M5, M4, W[2], W[1], W[0]], axis=1)
    assert mcat.shape == (E, MC_W)

    c5 = np.tile(cb5 @ W[4] + b[4], 8)  # [136]
    c4 = np.tile(cb4 @ W[3] + b[3], 2)  # [34]
    bias = np.zeros((68, 6), np.float32)
    bias[:68, 0] = c5[:68]
    bias[:68, 1] = c5[68:]
    bias[:34, 2] = c4
    bias[:17, 3] = b[2]
    bias[:17, 4] = b[1]
    bias[:17, 5] = b[0]

    mc = np.zeros((2, P, MC_WE), np.float32)
    mc[:, :, :MC_W] = mcat.reshape(2, P, MC_W)
    mc[0, :68, MC_W:] = bias

    in_maps = []
    for c in range(N_CORES):
        n, h = divmod(c, 2)
        parts = []
        for di in CORE_ORDER:
            lo = LOFFS[di] + h * HALF[di]
            parts.append(x[n, lo : lo + HALF[di]].T)
        xt = np.ascontiguousarray(np.concatenate(parts, axis=1), np.float32)
        in_maps.append({"x": xt.reshape(2, P, T_CORE), "mc": mc})
    return in_maps


def assemble_output(results):
    """Per-core result dicts -> full [N, T_TOK, V1] output."""
    out = np.empty((N_BATCH, int(TOKOFFS[-1]), V1), np.float32)
    for c in range(N_CORES):
        n, h = divmod(c, 2)
        r = results[c]
        per_depth = {
            4: np.concatenate([r["o5a"], r["o5b"]], axis=0),
            3: r["o4"],
            2: r["o3"],
            1: r["o2"],
            0: r["o1"],
        }
        for di in range(5):
            red = RED[di]
            th = HALF[di]  # latents in this half
            arr = per_depth[di]  # [red*V1, th]
            a = arr.reshape(red, V1, th).transpose(2, 0, 1).reshape(th * red, V1)
            t0 = int(TOKOFFS[di]) + h * (COUNTS[di] // 2)
            out[n, t0 : t0 + th * red] = a
    return out


def run_spmd(in_maps, trace=False, trace_kwargs=None):
    from concourse import bass_utils

    nc = _get_prog()
    return bass_utils.run_bass_kernel_spmd(
        nc,
        in_maps,
        list(range(N_CORES)),
        trace=trace,
        **(trace_kwargs or {}),
    )


def kernel(**inputs):
    res = run_spmd(shard_inputs(inputs))
    return assemble_output(res.results)
